# revision 11
# baseline (speedup 1.0000x reference)
"""Trainium2 Bass kernel for nn_DGN: batch-1 sequential GRU+memory decoder with
vocab-sharded logits GEMM (tensor-parallel over 8 NeuronCores).

Strategy:
  - The 60-step recurrence (GRU cell + memory-attention) only needs the small
    weights; it is replicated on all 8 cores (no collectives needed).
  - The heavy part (logits = tanh(...) @ Wo2.T + bo2 over VOCAB=100000) is
    deferred: per-step o1 pre-activations are accumulated in PSUM, then ONE
    batched GEMM [60,100] x [100+1, VOCAB/8] runs per core over its vocab
    shard.  This reads Wo2 exactly once (40MB total, 5MB/core) instead of 60
    times -- the memory roofline win.
  - log_softmax is only needed at the target token: per-core partial
    sum(exp(logits)) [60] plus the (replicated) target logits [60] are DMA'd
    out; the final log + sum (8*60 floats) is the host-side unshard step.
  - Embedding lookups (context 60x20 tokens, teacher-forced description
    tokens, and the 60 target rows of Wo2) are on-device indirect-DMA gathers
    from the full tables in HBM.
"""
import sys

for _p in ("/opt/trn_rl_repo",):
    if _p not in sys.path:
        sys.path.insert(0, _p)

import numpy as np

import concourse.bass as bass
import concourse.mybir as mybir
import concourse.tile as tile
from concourse.bass_utils import run_bass_kernel_spmd
from concourse.masks import make_identity
from concourse.vector_clock import ScopedClock

F32 = mybir.dt.float32
I32 = mybir.dt.int32
AF = mybir.ActivationFunctionType
OP = mybir.AluOpType
AX = mybir.AxisListType

VOCAB = 100000
EDIM = 100
HDIM = 100
T = 60           # decode steps == DESC_LEN
FACT = 20
CTX = 60
N_CORES = 8
VSH = VOCAB // N_CORES  # 12500 per-core vocab shard

# ---------------------------------------------------------------------------
# Workaround: this walrus build only accepts a single sync-wait on the CTRL
# (Drain) instruction Tile emits at kernel tail; split extra waits across
# chained drains (same engine => executes in order, semantics preserved).
_MAX_DRAIN_WAITS = 1


def _split_multi_waits(nc):
    """Walrus in this container accepts only one sync-wait per instruction.
    For any instruction carrying N>1 waits, emit N-1 same-engine NOPs right
    before it, each carrying one of the extra waits (same engine => sequencer
    order preserved => identical semantics)."""
    for f in nc.m.functions:
        for bb in f.blocks:
            insts = bb.instructions
            i = 0
            while i < len(insts):
                inst = insts[i]
                si = getattr(inst, "sync_info", None)
                if si is not None and si.on_wait and len(si.on_wait) > 1:
                    waits = list(si.on_wait)
                    inst.sync_info = mybir.SyncInfo(
                        on_wait=[waits[-1]], on_update=list(si.on_update or [])
                    )
                    new = []
                    for w in waits[:-1]:
                        ev = mybir.InstEventSemaphore(
                            name=nc.get_next_instruction_name(),
                            engine=inst.engine,
                            ins=[],
                            outs=[],
                            sync_info=mybir.SyncInfo(on_wait=[w], on_update=[]),
                        )
                        nc.register_instruction(ev, overwrite=True)
                        new.append(ev)
                    insts[i:i] = new
                    i += len(new) + 1
                else:
                    i += 1


def _patched_drain_and_barrier(self, tick_clock, wait_clock):
    _split_multi_waits(self.nc)
    drain_inst = self.nc.sync.drain()
    wait_clock.add_sem_waits(
        drain_inst.ins, ScopedClock({None: tick_clock.global_clock})
    )
    si = drain_inst.ins.sync_info
    if si is not None and si.on_wait and len(si.on_wait) > _MAX_DRAIN_WAITS:
        waits = list(si.on_wait)
        drain_inst.ins.sync_info = mybir.SyncInfo(
            on_wait=waits[:_MAX_DRAIN_WAITS], on_update=list(si.on_update or [])
        )
        for i in range(_MAX_DRAIN_WAITS, len(waits), _MAX_DRAIN_WAITS):
            extra = self.nc.sync.drain()
            extra.ins.sync_info = mybir.SyncInfo(
                on_wait=waits[i : i + _MAX_DRAIN_WAITS], on_update=[]
            )
    self.nc.all_engine_barrier()
    assert self.sems is not None
    popped = self.nc._tile_sem_poison_stack.pop()
    assert popped is self._sem_poison
    self.nc.clear_and_free_semaphores(list(self.sems.allocated().values()))
    self.nc.all_engine_barrier()


tile.TileContext._drain_and_barrier = _patched_drain_and_barrier

# Logits GEMM tiling: PSUM groups of 4 banks (4*512 fp32) per exp pass.
_GRP = 2048
_N_FULL_GRP = VSH // _GRP            # 6
_REM = VSH - _N_FULL_GRP * _GRP      # 212
_N_GRP = _N_FULL_GRP + (1 if _REM else 0)


def build_nc() -> bass.Bass:
    nc = bass.Bass()

    # ---- I/O declarations (same program on all 8 cores; data differs) ----
    ectx = nc.declare_dram_parameter("Ectx", [VOCAB, EDIM], F32, isOutput=False)
    edec = nc.declare_dram_parameter("Edec", [VOCAB, EDIM], F32, isOutput=False)
    w2rows = nc.declare_dram_parameter("W2rows", [VOCAB, HDIM + 1], F32, isOutput=False)
    wo2b = nc.declare_dram_parameter("Wo2b", [HDIM + 1, VSH], F32, isOutput=False)

    ctxidx = nc.declare_dram_parameter("ctxidx", [120, 10], I32, isOutput=False)
    toks = nc.declare_dram_parameter("toks", [T, 1], I32, isOutput=False)
    desc = nc.declare_dram_parameter("desc", [T, 1], I32, isOutput=False)

    rhs_ih_e = nc.declare_dram_parameter("rhs_ih_e", [100, 300], F32, isOutput=False)
    rhs_ih_m = nc.declare_dram_parameter("rhs_ih_m", [101, 300], F32, isOutput=False)
    rhs_hh = nc.declare_dram_parameter("rhs_hh", [101, 300], F32, isOutput=False)
    winitT = nc.declare_dram_parameter("WinitT", [100, 6000], F32, isOutput=False)
    w1tab = nc.declare_dram_parameter("W1Tab", [100, 200], F32, isOutput=False)
    w2c = nc.declare_dram_parameter("W2c", [100, 1], F32, isOutput=False)
    w3tm = nc.declare_dram_parameter("W3Tm", [101, 100], F32, isOutput=False)
    w3th = nc.declare_dram_parameter("W3Th", [100, 100], F32, isOutput=False)
    w3tc = nc.declare_dram_parameter("W3Tc", [100, 100], F32, isOutput=False)
    wo1th = nc.declare_dram_parameter("Wo1Th", [101, 100], F32, isOutput=False)
    wo1tc = nc.declare_dram_parameter("Wo1Tc", [100, 100], F32, isOutput=False)
    lbig = nc.declare_dram_parameter("lbig", [120, 100], F32, isOutput=False)
    blk = nc.declare_dram_parameter("blk", [120, 6], F32, isOutput=False)
    binit = nc.declare_dram_parameter("binit", [100, 1], F32, isOutput=False)
    b1 = nc.declare_dram_parameter("b1", [100, 1], F32, isOutput=False)

    out_se = nc.declare_dram_parameter("sumexp", [T, 1], F32, isOutput=True)
    out_tgt = nc.declare_dram_parameter("tgt", [1, T], F32, isOutput=True)

    with tile.TileContext(nc) as tc:
        with (
            tc.tile_pool(name="const", bufs=1) as cp,
            tc.tile_pool(name="work", bufs=2) as wp,
        ):
            # ---------------- constants into SBUF ----------------
            ident = cp.tile([128, 128], F32)
            make_identity(nc, ident[:])

            def load(name, ap, shape, dtype=F32):
                t_ = cp.tile(shape, dtype, tag=name)
                nc.sync.dma_start(out=t_[:], in_=ap[:])
                return t_

            rhs_ih_e_s = load("rhs_ih_e", rhs_ih_e, [100, 300])
            rhs_ih_m_s = load("rhs_ih_m", rhs_ih_m, [101, 300])
            rhs_hh_s = load("rhs_hh", rhs_hh, [101, 300])
            winitT_s = load("winitT", winitT, [100, 6000])
            w1tab_s = load("w1tab", w1tab, [100, 200])
            w2c_s = load("w2c", w2c, [100, 1])
            w3tm_s = load("w3tm", w3tm, [101, 100])
            w3th_s = load("w3th", w3th, [100, 100])
            w3tc_s = load("w3tc", w3tc, [100, 100])
            wo1th_s = load("wo1th", wo1th, [101, 100])
            wo1tc_s = load("wo1tc", wo1tc, [100, 100])
            lbig_s = load("lbig", lbig, [120, 100])
            blk_s = load("blk", blk, [120, 6])
            binit_s = load("binit", binit, [100, 1])
            b1_s = load("b1", b1, [100, 1])
            ctxidx_s = load("ctxidx", ctxidx, [120, 10], I32)
            toks_s = load("toks", toks, [T, 1], I32)
            desc_s = load("desc", desc, [T, 1], I32)

            # Wo2 shard prefetch (5MB) -- overlaps with the whole recurrence.
            wo2b_s = cp.tile([HDIM + 1, VSH], F32)
            for h in range(4):
                lo = h * (VSH // 4)
                hi = VSH if h == 3 else (h + 1) * (VSH // 4)
                nc.sync.dma_start(out=wo2b_s[:, lo:hi], in_=wo2b[:, lo:hi])

            # ---------------- phase A: facts & init ----------------
            po_ctx = tc.tile_pool(name="psO1", bufs=1, space="PSUM")
            ppo = po_ctx.__enter__()
            pa_ctx = tc.tile_pool(name="psA", bufs=1, space="PSUM")
            ppa = pa_ctx.__enter__()
            # gather context embeddings: 10 tiles of 120 rows (6 facts each)
            embg = cp.tile([120, 10, 100], F32)
            for j in range(10):
                nc.gpsimd.indirect_dma_start(
                    out=embg[:, j, :],
                    out_offset=None,
                    in_=ectx[:],
                    in_offset=bass.IndirectOffsetOnAxis(ap=ctxidx_s[:, j : j + 1], axis=0),
                )
            wemb = cp.tile([120, 10, 100], F32)
            for j in range(10):
                nc.vector.tensor_mul(wemb[:, j, :], embg[:, j, :], lbig_s[:])

            # facts_all[a, j, d] = facts[6j+a, d]  (one matmul, K=120)
            facts_all = cp.tile([6, 10, 100], F32)
            for hh in range(2):
                facts_ps = ppa.tile([6, 5, 100], F32, tag=f"factsps{hh}")
                nc.tensor.matmul(
                    facts_ps[:, :, :].rearrange("a j d -> a (j d)"),
                    lhsT=blk_s[:],
                    rhs=wemb[:, 5 * hh : 5 * hh + 5, :].rearrange("p j d -> p (j d)"),
                    start=True, stop=True)
                nc.vector.tensor_copy(facts_all[:, 5 * hh : 5 * hh + 5, :], facts_ps[:])

            ps_t = ppa.tile([101, 60], F32, tag="ps_t")
            for j in range(10):
                nc.tensor.transpose(ps_t[0:100, 6 * j : 6 * j + 6],
                                    facts_all[:, j, :], ident[0:6, 0:6])
            factsT = cp.tile([100, 60], F32)
            nc.vector.tensor_copy(factsT[:], ps_t[0:100, 0:60])

            # m0 = relu(Winit @ facts_flat + binit)
            ps_m0 = ppa.tile([100, 1], F32, tag="ps_m0")
            for i in range(60):
                nc.tensor.matmul(
                    ps_m0[:], lhsT=winitT_s[:, 100 * i : 100 * (i + 1)],
                    rhs=factsT[:, i : i + 1], start=(i == 0), stop=(i == 59),
                )
            mtil = cp.tile([101, 1], F32)
            nc.gpsimd.memset(mtil[:], 1.0)
            nc.scalar.activation(mtil[0:100, :], ps_m0[:], AF.Relu, bias=binit_s[:])

            htil = cp.tile([101, 1], F32)
            nc.gpsimd.memset(htil[:], 1.0)
            nc.gpsimd.memset(htil[0:100, :], 0.0)
            h_row = cp.tile([1, 100], F32)
            nc.gpsimd.memset(h_row[:], 0.0)

            # teacher-forced input embeddings, transposed -> [100, 60]
            edec_g = cp.tile([T, 100], F32)
            nc.gpsimd.indirect_dma_start(
                out=edec_g[:], out_offset=None, in_=edec[:],
                in_offset=bass.IndirectOffsetOnAxis(ap=toks_s[:, :1], axis=0),
            )
            ps_e = ppa.tile([101, 60], F32, tag="ps_t")
            nc.tensor.transpose(ps_e[0:100, 0:60], edec_g[:], ident[0:60, 0:60])
            edecT = cp.tile([100, T], F32)
            nc.vector.tensor_copy(edecT[:], ps_e[0:100, 0:60])

            # target rows of [Wo2 | bo2], transposed -> [101, 60]
            wg = cp.tile([T, HDIM + 1], F32)
            nc.gpsimd.indirect_dma_start(
                out=wg[:], out_offset=None, in_=w2rows[:],
                in_offset=bass.IndirectOffsetOnAxis(ap=desc_s[:, :1], axis=0),
            )
            ps_g = ppa.tile([101, 60], F32, tag="ps_t")
            nc.tensor.transpose(ps_g[:, 0:60], wg[:], ident[0:60, 0:60])
            gT = cp.tile([HDIM + 1, T], F32)
            nc.vector.tensor_copy(gT[:], ps_g[:, 0:60])

            # P3T / PO1T: fold facts into the c-branch of W3 / Wo1
            ps_p3 = ppa.tile([60, 100], F32, tag="ps_p3")
            nc.tensor.matmul(ps_p3[:], lhsT=factsT[:], rhs=w3tc_s[:], start=True, stop=True)
            p3T = cp.tile([60, 100], F32)
            nc.vector.tensor_copy(p3T[:], ps_p3[:])
            ps_po = ppa.tile([60, 100], F32, tag="ps_p3")
            nc.tensor.matmul(ps_po[:], lhsT=factsT[:], rhs=wo1tc_s[:], start=True, stop=True)
            po1T = cp.tile([60, 100], F32)
            nc.vector.tensor_copy(po1T[:], ps_po[:])

            # O1 pre-activation accumulator, one column per step
            o1ps = ppo.tile([100, T], F32)
            pa_ctx.__exit__(None, None, None)

            # ---------------- phase B: 60 recurrent steps ----------------
            with (
                tc.tile_pool(name="psB", bufs=1, space="PSUM") as pb,
                tc.tile_pool(name="wstep", bufs=2) as ws,
            ):
                for t in range(T):
                    # r,z pre-activations: gi+gh (+biases) in one PSUM group
                    ps_rz = pb.tile([1, 200], F32, tag="ps_rz")
                    nc.tensor.matmul(ps_rz[:], lhsT=edecT[:, t : t + 1],
                                     rhs=rhs_ih_e_s[:, 0:200], start=True, stop=False)
                    nc.tensor.matmul(ps_rz[:], lhsT=mtil[:],
                                     rhs=rhs_ih_m_s[:, 0:200], start=False, stop=False)
                    nc.tensor.matmul(ps_rz[:], lhsT=htil[:],
                                     rhs=rhs_hh_s[:, 0:200], start=False, stop=True)
                    # n-gate operands: inn (cols 0:100) and hn (cols 100:200)
                    ps_n = pb.tile([1, 200], F32, tag="ps_n")
                    nc.tensor.matmul(ps_n[:, 0:100], lhsT=edecT[:, t : t + 1],
                                     rhs=rhs_ih_e_s[:, 200:300], start=True, stop=False)
                    nc.tensor.matmul(ps_n[:, 0:100], lhsT=mtil[:],
                                     rhs=rhs_ih_m_s[:, 200:300], start=False, stop=True)
                    nc.tensor.matmul(ps_n[:, 100:200], lhsT=htil[:],
                                     rhs=rhs_hh_s[:, 200:300], start=True, stop=True)

                    trz = ws.tile([1, 200], F32, tag="trz")
                    nc.scalar.activation(trz[:], ps_rz[:], AF.Tanh, scale=0.5)

                    # rhn = 0.5*(trz_r + 1) * hn ; npre = inn + rhn
                    u = ws.tile([1, 100], F32, tag="u")
                    nc.vector.scalar_tensor_tensor(
                        u[:], trz[:, 0:100], 1.0, ps_n[:, 100:200], OP.add, OP.mult)
                    npre = ws.tile([1, 100], F32, tag="npre")
                    nc.vector.scalar_tensor_tensor(
                        npre[:], u[:], 0.5, ps_n[:, 0:100], OP.mult, OP.add)
                    tn = ws.tile([1, 100], F32, tag="tn")
                    nc.scalar.activation(tn[:], npre[:], AF.Tanh)

                    # h_new = 0.5*((tn + h) + trz_z * (h - tn))
                    d_ = ws.tile([1, 100], F32, tag="d_")
                    nc.vector.scalar_tensor_tensor(
                        d_[:], tn[:], -1.0, h_row[:], OP.mult, OP.add)
                    e_ = ws.tile([1, 100], F32, tag="e_")
                    nc.vector.tensor_mul(e_[:], trz[:, 100:200], d_[:])
                    f_ = ws.tile([1, 100], F32, tag="f_")
                    nc.vector.tensor_add(f_[:], tn[:], h_row[:])
                    g2 = ws.tile([1, 100], F32, tag="g2")
                    nc.vector.tensor_add(g2[:], e_[:], f_[:])
                    nc.vector.tensor_scalar_mul(h_row[:], g2[:], 0.5)

                    ps_h = pb.tile([100, 1], F32, tag="ps_h")
                    nc.tensor.transpose(ps_h[:], h_row[:], ident[0:1, 0:1])
                    nc.vector.tensor_copy(htil[0:100, :], ps_h[:])

                    # memory attention: |facts - h|, |facts - m| (abs via max(p,-p))
                    pA = ws.tile([100, 60], F32, tag="pA")
                    nc.vector.tensor_scalar(pA[:], factsT[:], htil[0:100, :], None,
                                            OP.subtract)
                    qA = ws.tile([100, 60], F32, tag="qA")
                    nc.vector.tensor_scalar_mul(qA[:], pA[:], -1.0)
                    zA = ws.tile([100, 60], F32, tag="zA")
                    nc.vector.tensor_tensor(zA[:], pA[:], qA[:], OP.max)
                    pB = ws.tile([100, 60], F32, tag="pB")
                    nc.vector.tensor_scalar(pB[:], factsT[:], mtil[0:100, :], None,
                                            OP.subtract)
                    qB = ws.tile([100, 60], F32, tag="qB")
                    nc.vector.tensor_scalar_mul(qB[:], pB[:], -1.0)
                    zB = ws.tile([100, 60], F32, tag="zB")
                    nc.vector.tensor_tensor(zB[:], pB[:], qB[:], OP.max)
                    psA = pb.tile([100, 60], F32, tag="psA")
                    nc.tensor.matmul(psA[:], lhsT=w1tab_s[:, 0:100], rhs=zA[:],
                                     start=True, stop=False)
                    nc.tensor.matmul(psA[:], lhsT=w1tab_s[:, 100:200], rhs=zB[:],
                                     start=False, stop=True)
                    a_t = ws.tile([100, 60], F32, tag="a_t")
                    nc.scalar.activation(a_t[:], psA[:], AF.Tanh, bias=b1_s[:])

                    psg = pb.tile([1, 60], F32, tag="psg")
                    nc.tensor.matmul(psg[:], lhsT=w2c_s[:], rhs=a_t[:], start=True, stop=True)
                    eg = ws.tile([1, 60], F32, tag="eg")
                    se = ws.tile([1, 1], F32, tag="se")
                    nc.scalar.activation(eg[:], psg[:], AF.Exp, accum_out=se[:])
                    inv = ws.tile([1, 1], F32, tag="inv")
                    nc.vector.reciprocal(inv[:], se[:])
                    gn = ws.tile([1, 60], F32, tag="gn")
                    nc.vector.tensor_scalar_mul(gn[:], eg[:], inv[:])
                    ps_gt = pb.tile([60, 1], F32, tag="ps_gt")
                    nc.tensor.transpose(ps_gt[:], gn[:], ident[0:1, 0:1])
                    gnT = ws.tile([60, 1], F32, tag="gnT")
                    nc.vector.tensor_copy(gnT[:], ps_gt[:])

                    # m_new = relu(W3 @ [m; c; h] + b3)
                    psm = pb.tile([100, 1], F32, tag="psm")
                    nc.tensor.matmul(psm[:], lhsT=w3tm_s[:], rhs=mtil[:],
                                     start=True, stop=False)
                    nc.tensor.matmul(psm[:], lhsT=p3T[:], rhs=gnT[:],
                                     start=False, stop=False)
                    nc.tensor.matmul(psm[:], lhsT=w3th_s[:], rhs=htil[0:100, :],
                                     start=False, stop=True)
                    nc.scalar.activation(mtil[0:100, :], psm[:], AF.Relu)

                    # o1 pre-activation column t (tanh deferred to phase C)
                    nc.tensor.matmul(o1ps[:, t : t + 1], lhsT=wo1th_s[:], rhs=htil[:],
                                     start=True, stop=False)
                    nc.tensor.matmul(o1ps[:, t : t + 1], lhsT=po1T[:], rhs=gnT[:],
                                     start=False, stop=True)

            # ---------------- phase C: batched logits GEMM ----------------
            o1T = cp.tile([HDIM + 1, T], F32)
            nc.gpsimd.memset(o1T[:], 1.0)
            nc.scalar.activation(o1T[0:100, :], o1ps[:], AF.Tanh)
            po_ctx.__exit__(None, None, None)

            # target logits: tgt[t] = sum_d o1b[d,t] * gT[d,t]
            ones101 = cp.tile([101, 1], F32)
            nc.gpsimd.memset(ones101[:], 1.0)
            prod = cp.tile([HDIM + 1, T], F32)
            nc.vector.tensor_mul(prod[:], o1T[:], gT[:])
            with tc.tile_pool(name="psT", bufs=1, space="PSUM") as ppt:
                ps_tg = ppt.tile([1, 60], F32, tag="ps_tg")
                nc.tensor.matmul(ps_tg[:], lhsT=ones101[:], rhs=prod[:], start=True, stop=True)
                tgt_s = cp.tile([1, T], F32)
                nc.vector.tensor_copy(tgt_s[:], ps_tg[:])
            nc.sync.dma_start(out=out_tgt[:], in_=tgt_s[:])

            seps = cp.tile([T, _N_GRP], F32)
            with (
                tc.tile_pool(name="psC", bufs=2, space="PSUM") as pc,
                tc.tile_pool(name="wC", bufs=2) as wc,
            ):
                for g in range(_N_GRP):
                    lo = g * _GRP
                    hi = min(VSH, lo + _GRP)
                    n = hi - lo
                    psl = pc.tile([T, _GRP], F32, tag="psl")
                    for c in range(0, n, 512):
                        ce = min(n, c + 512)
                        nc.tensor.matmul(
                            psl[:, c:ce], lhsT=o1T[:], rhs=wo2b_s[:, lo + c : lo + ce],
                            start=True, stop=True,
                        )
                    scr = wc.tile([T, _GRP], F32, tag="scr")
                    nc.scalar.activation(scr[:, 0:n], psl[:, 0:n], AF.Exp,
                                         accum_out=seps[:, g : g + 1])

            se_tot = cp.tile([T, 1], F32)
            nc.vector.reduce_sum(se_tot[:], seps[:], axis=AX.X)
            nc.sync.dma_start(out=out_se[:], in_=se_tot[:])

    return nc


def _host_inputs(inputs: dict) -> list[dict]:
    """Build the 8 per-core input maps from the full problem inputs."""
    f32 = np.float32
    ctx = np.asarray(inputs["context"]).astype(np.int32)        # [60,20]
    descv = np.asarray(inputs["description"]).astype(np.int32)  # [60]
    Ectx = np.ascontiguousarray(np.asarray(inputs["E_ctx"], dtype=f32))
    Edec = np.ascontiguousarray(np.asarray(inputs["E_dec"], dtype=f32))
    W1 = np.asarray(inputs["W1"], dtype=f32)
    b1v = np.asarray(inputs["b1"], dtype=f32)
    W2 = np.asarray(inputs["W2"], dtype=f32)
    W3 = np.asarray(inputs["W3"], dtype=f32)
    b3v = np.asarray(inputs["b3"], dtype=f32)
    Wih = np.asarray(inputs["Wih"], dtype=f32)
    Whh = np.asarray(inputs["Whh"], dtype=f32)
    bih = np.asarray(inputs["bih"], dtype=f32)
    bhh = np.asarray(inputs["bhh"], dtype=f32)
    Winit = np.asarray(inputs["Winit"], dtype=f32)
    binitv = np.asarray(inputs["binit"], dtype=f32)
    Wo1 = np.asarray(inputs["Wo1"], dtype=f32)
    bo1 = np.asarray(inputs["bo1"], dtype=f32)
    Wo2 = np.asarray(inputs["Wo2"], dtype=f32)
    bo2 = np.asarray(inputs["bo2"], dtype=f32)

    # index tensors
    ctxidx = np.zeros((120, 10), np.int32)
    for p in range(120):
        for j in range(10):
            ctxidx[p, j] = ctx[6 * j + p // 20, p % 20]
    toks = np.concatenate([[1], descv[:-1]]).astype(np.int32).reshape(T, 1)
    descc = descv.reshape(T, 1)

    # positional encoder weights
    s = np.arange(FACT, dtype=f32) / (FACT - 1)
    e = np.arange(EDIM, dtype=f32) / (EDIM - 1)
    l = 1.0 - s[:, None] - e[None, :] * (1.0 - 2.0 * s[:, None])   # [20,100]
    lbig = np.tile(l, (6, 1)).astype(f32)                          # [120,100]
    blk = np.zeros((120, 6), f32)
    for p in range(120):
        blk[p, p // 20] = 1.0

    WihT = Wih.T.copy()   # [200,300]
    bias1 = np.concatenate([bih[0:100] + bhh[0:100],
                            bih[100:200] + bhh[100:200],
                            bih[200:300]]).astype(f32)
    rhs_ih_e = np.ascontiguousarray(WihT[0:100])
    rhs_ih_m = np.ascontiguousarray(np.vstack([WihT[100:200], bias1[None, :]]))
    bias2 = np.concatenate([np.zeros(200, f32), bhh[200:300]]).astype(f32)
    rhs_hh = np.ascontiguousarray(np.vstack([Whh.T, bias2[None, :]]))

    WinitT = np.ascontiguousarray(
        Winit.T.reshape(60, 100, 100).transpose(1, 0, 2).reshape(100, 6000))
    W1T = W1.T
    W1Tab = np.ascontiguousarray(np.concatenate([W1T[0:100], W1T[100:200]], axis=1))
    W2c = np.ascontiguousarray(W2.T)  # [100,1]
    W3T = W3.T
    W3Tm = np.ascontiguousarray(np.vstack([W3T[0:100], b3v[None, :]]))
    W3Th = np.ascontiguousarray(W3T[200:300])
    W3Tc = np.ascontiguousarray(W3T[100:200])
    Wo1T = Wo1.T
    Wo1Th = np.ascontiguousarray(np.vstack([Wo1T[0:100], bo1[None, :]]))
    Wo1Tc = np.ascontiguousarray(Wo1T[100:200])

    W2rows = np.ascontiguousarray(np.concatenate([Wo2, bo2[:, None]], axis=1))
    Wo2bT = np.ascontiguousarray(np.vstack([Wo2.T, bo2[None, :]]))  # [101, V]

    shared = dict(
        Ectx=Ectx, Edec=Edec, W2rows=W2rows,
        ctxidx=ctxidx, toks=toks, desc=descc,
        rhs_ih_e=rhs_ih_e, rhs_ih_m=rhs_ih_m, rhs_hh=rhs_hh,
        WinitT=WinitT, W1Tab=W1Tab, W2c=W2c, W3Tm=W3Tm, W3Th=W3Th, W3Tc=W3Tc,
        Wo1Th=Wo1Th, Wo1Tc=Wo1Tc, lbig=lbig, blk=blk,
        binit=binitv.reshape(100, 1), b1=b1v.reshape(100, 1),
    )
    maps = []
    for r in range(N_CORES):
        m = dict(shared)
        m["Wo2b"] = np.ascontiguousarray(Wo2bT[:, r * VSH : (r + 1) * VSH])
        maps.append(m)
    return maps


_NC_CACHE = {}


def get_nc() -> bass.Bass:
    if "nc" not in _NC_CACHE:
        _NC_CACHE["nc"] = build_nc()
    return _NC_CACHE["nc"]


def kernel(**inputs) -> np.ndarray:
    nc = get_nc()
    in_maps = _host_inputs(inputs)
    res = run_bass_kernel_spmd(nc, in_maps, list(range(N_CORES)), trace=False)
    ses = np.stack([res.results[r]["sumexp"][:, 0] for r in range(N_CORES)])  # [8,60]
    tgt = res.results[0]["tgt"][0]                                            # [60]
    lse = np.log(ses.astype(np.float64).sum(axis=0))
    loss = (lse - tgt.astype(np.float64)).sum()
    return np.asarray(loss, dtype=np.float32)
